# revision 1
# baseline (speedup 1.0000x reference)
"""Trainium2 kernel for the FEM kinematic (strain) layer.

Reference computation:
    disp = inputs[:, elem_nodes]                      # [B, E, 8, 2]
    dd   = einsum('egkl,bekn->begnl', shpdx, disp)    # [B, E, 9, 2, 2]
    out  = stack([dd[...,0,0], dd[...,1,1],
                  0.5*(dd[...,0,1] + dd[...,1,0])])   # [B, E*9, 3]

Sharding: elements split across 8 NeuronCores.  The host resolves the
element->node indirection (index marshalling) and ships each core an
element-major displacement block; the device streams shpdx + disp and
computes the strain products with DVE, using the identity
    S1*u + S0*v = (S0+S1)*(u+v) - S0*u - S1*v
so only 3 elementwise products are needed per (element, gauss point).
"""

import sys
import numpy as np

sys.path.insert(0, "/opt/trn_rl_repo")

import concourse.bass as bass
import concourse.bacc as bacc
import concourse.mybir as mybir
import concourse.tile as tile
from concourse.bass_utils import run_bass_kernel_spmd

B = 4
N_NODES = 1_000_000
N_ELEM = 500_000
N_GP = 9
N_EN = 8
N_CORES = 8

E_CORE = N_ELEM // N_CORES            # 62500 elements per core
P = 128                               # SBUF partitions
C = 16                                # elements per partition per chunk
CHUNK = P * C                         # 2048 elements per chunk
N_CHUNKS = -(-E_CORE // CHUNK)        # 31
E_PAD = N_CHUNKS * CHUNK              # 63488 (988 pad elements)

_compiled = None


def _build_program():
    nc = bacc.Bacc("TRN2", target_bir_lowering=False, debug=False)
    f32 = mybir.dt.float32

    # [E_PAD, 144] f32: per element (g, k, l) row-major
    s_d = nc.dram_tensor("shp", [E_PAD, 144], f32, kind="ExternalInput").ap()
    # [E_PAD, 64] f32: per element (k, b, n) row-major
    d_d = nc.dram_tensor("disp", [E_PAD, 64], f32, kind="ExternalInput").ap()
    # [B, E_PAD*9, 3] f32
    o_d = nc.dram_tensor("out", [B, E_PAD * 9, 3], f32, kind="ExternalOutput").ap()

    s_v = s_d.rearrange("(n p c) f -> n p (c f)", p=P, c=C)
    d_v = d_d.rearrange("(n p c) f -> n p (c f)", p=P, c=C)
    # out view per (b, chunk): [P, C*27]
    o_v = o_d.rearrange("b (n p x) three -> b n p (x three)", p=P, x=C * 9)

    with tile.TileContext(nc) as tc:
        with (
            tc.tile_pool(name="io", bufs=4) as io_pool,
            tc.tile_pool(name="tmp", bufs=3) as tmp_pool,
        ):
            for i in range(N_CHUNKS):
                S = io_pool.tile([P, C * 144], f32, tag="S")
                D = io_pool.tile([P, C * 64], f32, tag="D")
                nc.sync.dma_start(out=S[:], in_=s_v[i])
                nc.sync.dma_start(out=D[:], in_=d_v[i])

                Sr = S[:].rearrange("p (c g k l) -> p c g k l", c=C, g=9, k=8, l=2)
                Dr = D[:].rearrange("p (c k b n) -> p c k b n", c=C, k=8, b=B, n=2)

                # A = S0 + S1, contiguous [p, (c g k)]
                A = tmp_pool.tile([P, C * 72], f32, tag="A")
                Av = A[:].rearrange("p (c g k) -> p c g k", c=C, g=9)
                nc.vector.tensor_tensor(
                    out=Av, in0=Sr[:, :, :, :, 0], in1=Sr[:, :, :, :, 1],
                    op=mybir.AluOpType.add,
                )

                O = io_pool.tile([P, B * C * 27], f32, tag="O")
                Ov = O[:].rearrange("p (b c g t) -> p b c g t", b=B, c=C, g=9, t=3)

                for b in range(B):
                    u = Dr[:, :, :, b, 0]          # [p, C, 8]
                    v = Dr[:, :, :, b, 1]

                    W = tmp_pool.tile([P, C * 8], f32, tag="W")
                    Wv = W[:].rearrange("p (c k) -> p c k", c=C)
                    nc.gpsimd.tensor_tensor(
                        out=Wv, in0=u, in1=v, op=mybir.AluOpType.add
                    )

                    # broadcast displacement over g: [p, C, 1, 8] -> [p, C, 9, 8]
                    ub = u[:, :, None, :].to_broadcast([P, C, 9, 8])
                    vb = v[:, :, None, :].to_broadcast([P, C, 9, 8])
                    wb = Wv[:, :, None, :].to_broadcast([P, C, 9, 8])

                    T0 = tmp_pool.tile([P, C * 72], f32, tag="T")
                    T1 = tmp_pool.tile([P, C * 72], f32, tag="T")
                    T2 = tmp_pool.tile([P, C * 72], f32, tag="T")
                    T0v = T0[:].rearrange("p (c g k) -> p c g k", c=C, g=9)
                    T1v = T1[:].rearrange("p (c g k) -> p c g k", c=C, g=9)
                    T2v = T2[:].rearrange("p (c g k) -> p c g k", c=C, g=9)

                    nc.vector.tensor_tensor(
                        out=T0v, in0=Sr[:, :, :, :, 0], in1=ub,
                        op=mybir.AluOpType.mult,
                    )
                    nc.vector.tensor_tensor(
                        out=T1v, in0=Sr[:, :, :, :, 1], in1=vb,
                        op=mybir.AluOpType.mult,
                    )
                    nc.vector.tensor_tensor(
                        out=T2v, in0=Av, in1=wb, op=mybir.AluOpType.mult,
                    )

                    # xx / yy land directly in the (strided) output staging
                    nc.vector.reduce_sum(
                        out=Ov[:, b, :, :, 0], in_=T0v, axis=mybir.AxisListType.X
                    )
                    nc.vector.reduce_sum(
                        out=Ov[:, b, :, :, 1], in_=T1v, axis=mybir.AxisListType.X
                    )

                    R = tmp_pool.tile([P, C * 9], f32, tag="R")
                    Rv = R[:].rearrange("p (c g) -> p c g", c=C)
                    nc.vector.reduce_sum(out=Rv, in_=T2v, axis=mybir.AxisListType.X)
                    nc.gpsimd.tensor_tensor(
                        out=Rv, in0=Rv, in1=Ov[:, b, :, :, 0],
                        op=mybir.AluOpType.subtract,
                    )
                    nc.gpsimd.tensor_tensor(
                        out=Rv, in0=Rv, in1=Ov[:, b, :, :, 1],
                        op=mybir.AluOpType.subtract,
                    )
                    nc.scalar.activation(
                        out=Ov[:, b, :, :, 2], in_=Rv,
                        func=mybir.ActivationFunctionType.Copy, scale=0.5,
                    )

                for b in range(B):
                    nc.sync.dma_start(
                        out=o_v[b, i],
                        in_=O[:, b * C * 27:(b + 1) * C * 27],
                    )

    nc.compile()
    return nc


def _get_program():
    global _compiled
    if _compiled is None:
        _compiled = _build_program()
    return _compiled


def kernel(inputs, shpdx, elem_nodes, _want_trace=False):
    nc = _get_program()

    # Host-side index marshalling: resolve element->node indirection and
    # build per-core element-major blocks.
    in_maps = []
    for c in range(N_CORES):
        sl = slice(c * E_CORE, (c + 1) * E_CORE)
        en = elem_nodes[sl]                                   # [E, 8]
        disp = inputs[:, en]                                  # [B, E, 8, 2]
        dispc = np.ascontiguousarray(disp.transpose(1, 2, 0, 3))  # [E, 8, B, 2]
        dispc = dispc.reshape(E_CORE, 64)
        dpad = np.zeros((E_PAD, 64), np.float32)
        dpad[:E_CORE] = dispc
        spad = np.zeros((E_PAD, 144), np.float32)
        spad[:E_CORE] = shpdx[sl].reshape(E_CORE, 144)
        in_maps.append({"shp": spad, "disp": dpad})

    core_ids = list(range(N_CORES))
    res = run_bass_kernel_spmd(nc, in_maps, core_ids, trace=_want_trace)

    outs = []
    for c in range(N_CORES):
        o = res.results[c]["out"]                             # [B, E_PAD*9, 3]
        outs.append(o[:, :E_CORE * 9, :])
    full = np.concatenate(outs, axis=1)                       # [B, N_ELEM*9, 3]
    if _want_trace:
        return full, res
    return full



# revision 5
# speedup vs baseline: 1.9799x; 1.9799x over previous
"""Trainium2 kernel for the FEM kinematic (strain) layer.

Reference computation:
    disp = inputs[:, elem_nodes]                      # [B, E, 8, 2]
    dd   = einsum('egkl,bekn->begnl', shpdx, disp)    # [B, E, 9, 2, 2]
    out  = stack([dd[...,0,0], dd[...,1,1],
                  0.5*(dd[...,0,1] + dd[...,1,0])])   # [B, E*9, 3]

Sharding: elements split across 8 NeuronCores.  The host resolves the
element->node indirection and ships each core bf16 element-major blocks:
  S' = (s0, s1, (s0+s1)/2) per (e,g,k)   and   D' = (u, v, u+v) per (e,b,k).
The device computes, per (b,e,g), the three 8-term dot products
  P0 = s0.u, P1 = s1.v, P2 = 0.5*(s0+s1).(u+v)
via one bf16 tensor_tensor multiply (DVE 2x mode) + a pairwise add tree,
then combines  xx = P0, yy = P1, xy = P2 - 0.5*P0 - 0.5*P1.
Tree/combine work is split between the Vector and GpSimd engines.
"""

import sys
import numpy as np

sys.path.insert(0, "/opt/trn_rl_repo")

import concourse.bass as bass
import concourse.bacc as bacc
import concourse.mybir as mybir
import concourse.tile as tile
from concourse.bass_utils import run_bass_kernel_spmd

import ml_dtypes

BF16 = ml_dtypes.bfloat16

B = 4
N_NODES = 1_000_000
N_ELEM = 500_000
N_GP = 9
N_EN = 8
N_CORES = 8

E_CORE = N_ELEM // N_CORES            # 62500 elements per core
P = 128                               # SBUF partitions
C = 24                                # elements per partition per chunk
CHUNK = P * C                         # 3072 elements per chunk
N_CHUNKS = -(-E_CORE // CHUNK)        # 21
E_PAD = N_CHUNKS * CHUNK              # 64512

S_EL = N_GP * 3 * N_EN                # 216: per element (g, t, k)
D_EL = B * 3 * N_EN                   # 96:  per element (b, t, k)
O_EL = B * N_GP * 3                   # 108: per element (b, g, t)

_compiled = None


def _build_program():
    nc = bacc.Bacc("TRN2", target_bir_lowering=False, debug=False)
    f32 = mybir.dt.float32
    bf16 = mybir.dt.bfloat16

    s_d = nc.dram_tensor("sp", [E_PAD, S_EL], bf16, kind="ExternalInput").ap()
    d_d = nc.dram_tensor("dp", [E_PAD, D_EL], bf16, kind="ExternalInput").ap()
    o_d = nc.dram_tensor("out", [B, E_PAD * N_GP, 3], bf16, kind="ExternalOutput").ap()

    s_v = s_d.rearrange("(n p c) f -> n p (c f)", p=P, c=C)
    d_v = d_d.rearrange("(n p c) f -> n p (c f)", p=P, c=C)
    # per chunk: [p, b, (c g t)] so src/dst iterate (p, b, run)
    o_v = o_d.rearrange("b (n p c g) t -> n p b (c g t)", p=P, c=C, g=N_GP)

    with tile.TileContext(nc) as tc:
        with (
            tc.tile_pool(name="io", bufs=3) as io_pool,
            tc.tile_pool(name="mid", bufs=2) as mid_pool,
        ):
            for i in range(N_CHUNKS):
                S = io_pool.tile([P, C * S_EL], bf16, tag="S")
                D = io_pool.tile([P, C * D_EL], bf16, tag="D")
                nc.sync.dma_start(out=S[:], in_=s_v[i])
                nc.sync.dma_start(out=D[:], in_=d_v[i])

                Sv = S[:].rearrange("p (c g t k) -> p c g t k", c=C, g=N_GP, t=3)
                Dv = D[:].rearrange("p (c b t k) -> p c b t k", c=C, b=B, t=3)

                T = mid_pool.tile([P, C * 864], bf16, tag="T")
                Tv = T[:].rearrange("p (c b g t k) -> p c b g t k", c=C, b=B, g=N_GP, t=3)

                # products: T[c,b,g,t,k] = S'[c,g,t,k] * D'[c,b,t,k]
                # one op per b keeps APs at 4 free dims (DVE 2x: k contiguous)
                for b in range(B):
                    db = Dv[:, :, b]                       # [p, c, t, k]
                    dbg = db[:, :, None, :, :].to_broadcast([P, C, N_GP, 3, N_EN])
                    nc.vector.tensor_tensor(
                        out=Tv[:, :, b], in0=Sv, in1=dbg, op=mybir.AluOpType.mult
                    )

                U1 = mid_pool.tile([P, C * 432], bf16, tag="U1")
                U1v = U1[:].rearrange(
                    "p (c b g t k) -> p c b g t k", c=C, b=B, g=N_GP, t=3
                )
                # L1: sum k-halves (k: 8 -> 4); split across DVE / GpSimd
                for b in range(B):
                    eng = nc.gpsimd if b == 0 else nc.vector
                    eng.tensor_tensor(
                        out=U1v[:, :, b],
                        in0=Tv[:, :, b, :, :, 0:4],
                        in1=Tv[:, :, b, :, :, 4:8],
                        op=mybir.AluOpType.add,
                    )

                U2 = mid_pool.tile([P, C * 216], bf16, tag="U2")
                U2v = U2[:].rearrange(
                    "p (c b g t k) -> p c b g t k", c=C, b=B, g=N_GP, t=3
                )
                # L2: k: 4 -> 2, all on GpSimd
                for b in range(B):
                    nc.gpsimd.tensor_tensor(
                        out=U2v[:, :, b],
                        in0=U1v[:, :, b, :, :, 0:2],
                        in1=U1v[:, :, b, :, :, 2:4],
                        op=mybir.AluOpType.add,
                    )

                O = io_pool.tile([P, B * C * 27], bf16, tag="O")
                # per-partition layout [b][c][g][t], viewed with (c, b, g) order
                # to match the U2 slices' iteration order
                Ocb = O[:].rearrange("p (b c g t) -> p c b g t", b=B, c=C, g=N_GP)

                TMP = mid_pool.tile([P, C * B * N_GP], bf16, tag="TMP")
                TMPv = TMP[:].rearrange("p (c b g) -> p c b g", c=C, b=B)

                # L3 + strain combine (all-b single ops, 1x mode):
                #   xx = U2[t0,0]+U2[t0,1] ; yy = U2[t1,...] ; p2 = U2[t2,...]
                #   xy = p2 - 0.5 xx - 0.5 yy
                xx_o = Ocb[:, :, :, :, 0]
                yy_o = Ocb[:, :, :, :, 1]
                xy_o = Ocb[:, :, :, :, 2]
                nc.vector.tensor_tensor(
                    out=xx_o, in0=U2v[:, :, :, :, 0, 0],
                    in1=U2v[:, :, :, :, 0, 1], op=mybir.AluOpType.add,
                )
                nc.vector.tensor_tensor(
                    out=yy_o, in0=U2v[:, :, :, :, 1, 0],
                    in1=U2v[:, :, :, :, 1, 1], op=mybir.AluOpType.add,
                )
                nc.vector.tensor_tensor(
                    out=TMPv, in0=U2v[:, :, :, :, 2, 0],
                    in1=U2v[:, :, :, :, 2, 1], op=mybir.AluOpType.add,
                )
                # xy = (xx * -0.5 + p2) ; then xy = (yy * -0.5 + xy)
                # TensorScalarPtr APs are limited to 2 free dims -> per-b ops
                for b in range(B):
                    nc.vector.scalar_tensor_tensor(
                        out=TMPv[:, :, b], in0=xx_o[:, :, b], scalar=-0.5,
                        in1=TMPv[:, :, b],
                        op0=mybir.AluOpType.mult, op1=mybir.AluOpType.add,
                    )
                    nc.vector.scalar_tensor_tensor(
                        out=xy_o[:, :, b], in0=yy_o[:, :, b], scalar=-0.5,
                        in1=TMPv[:, :, b],
                        op0=mybir.AluOpType.mult, op1=mybir.AluOpType.add,
                    )

                nc.sync.dma_start(
                    out=o_v[i],
                    in_=O[:].rearrange("p (b f) -> p b f", b=B),
                )

    nc.compile()
    return nc


def _get_program():
    global _compiled
    if _compiled is None:
        _compiled = _build_program()
    return _compiled


def _marshal_core(inputs, shpdx, elem_nodes, c):
    """Build the per-core bf16 S'/D' blocks."""
    sl = slice(c * E_CORE, (c + 1) * E_CORE)
    en = elem_nodes[sl]                                   # [E, 8]
    disp = inputs[:, en]                                  # [B, E, 8, 2]
    u = disp[..., 0]                                      # [B, E, 8]
    v = disp[..., 1]
    w = u + v
    # D'[e, b, t, k] with t = (u, v, w)
    dstk = np.stack([u, v, w], axis=2)                    # [B, E, 3, 8]
    dstk = dstk.transpose(1, 0, 2, 3)                     # [E, B, 3, 8]
    dpad = np.zeros((E_PAD, D_EL), BF16)
    dpad[:E_CORE] = dstk.reshape(E_CORE, D_EL).astype(BF16)

    sx = shpdx[sl]                                        # [E, 9, 8, 2]
    s0 = sx[..., 0]
    s1 = sx[..., 1]
    a = 0.5 * (s0 + s1)
    sstk = np.stack([s0, s1, a], axis=2)                  # [E, 9, 3, 8]
    spad = np.zeros((E_PAD, S_EL), BF16)
    spad[:E_CORE] = sstk.reshape(E_CORE, S_EL).astype(BF16)
    return {"sp": spad, "dp": dpad}


def kernel(inputs, shpdx, elem_nodes, _want_trace=False):
    nc = _get_program()

    in_maps = [
        _marshal_core(inputs, shpdx, elem_nodes, c) for c in range(N_CORES)
    ]

    core_ids = list(range(N_CORES))
    res = run_bass_kernel_spmd(nc, in_maps, core_ids, trace=_want_trace)

    outs = []
    for c in range(N_CORES):
        o = res.results[c]["out"]                         # [B, E_PAD*9, 3] bf16
        outs.append(np.asarray(o[:, :E_CORE * N_GP, :], np.float32))
    full = np.concatenate(outs, axis=1)                   # [B, N_ELEM*9, 3]
    if _want_trace:
        return full, res
    return full


# revision 15
# speedup vs baseline: 2.0438x; 1.0323x over previous
"""Trainium2 kernel for the FEM kinematic (strain) layer.

Reference computation:
    disp = inputs[:, elem_nodes]                      # [B, E, 8, 2]
    dd   = einsum('egkl,bekn->begnl', shpdx, disp)    # [B, E, 9, 2, 2]
    out  = stack([dd[...,0,0], dd[...,1,1],
                  0.5*(dd[...,0,1] + dd[...,1,0])])   # [B, E*9, 3]

Sharding: elements split across 8 NeuronCores.  The host resolves the
element->node indirection and ships each core bf16 element-major blocks:
  S' = (s0, s1, (s0+s1)/2) per (e,g,k)   and   D' = (u, v, u+v) per (e,b,k).
The device computes, per (b,e,g), the three 8-term dot products
  P0 = s0.u, P1 = s1.v, P2 = 0.5*(s0+s1).(u+v)
via one bf16 tensor_tensor multiply (DVE 2x mode) + a pairwise add tree,
then combines  xx = P0, yy = P1, xy = P2 - 0.5*P0 - 0.5*P1.
Tree/combine work is split between the Vector and GpSimd engines.
"""

import sys
import numpy as np

sys.path.insert(0, "/opt/trn_rl_repo")

import concourse.bass as bass
import concourse.bacc as bacc
import concourse.mybir as mybir
import concourse.tile as tile
from concourse.bass_utils import run_bass_kernel_spmd

import ml_dtypes

BF16 = ml_dtypes.bfloat16

B = 4
N_NODES = 1_000_000
N_ELEM = 500_000
N_GP = 9
N_EN = 8
N_CORES = 8

E_CORE = N_ELEM // N_CORES            # 62500 elements per core
P = 128                               # SBUF partitions
C = 36                                # elements per partition per chunk
IO_BUFS = 3
MID_BUFS = 1                          # per-b tile tags already pipeline
POOL_L1 = 1                           # b's of L1 on GpSimd (rest DVE)
DVE_L2 = 0                            # b's of L2 on DVE (rest GpSimd)
CHUNK = P * C                         # 4608 elements per chunk
N_CHUNKS = -(-E_CORE // CHUNK)        # 14
E_PAD = N_CHUNKS * CHUNK              # 64512

S_EL = N_GP * 3 * N_EN                # 216: per element (g, t, k)
D_EL = B * 3 * N_EN                   # 96:  per element (b, t, k)
O_EL = B * N_GP * 3                   # 108: per element (b, g, t)

_compiled = None


def _build_program():
    nc = bacc.Bacc("TRN2", target_bir_lowering=False, debug=False)
    f32 = mybir.dt.float32
    bf16 = mybir.dt.bfloat16

    s_d = nc.dram_tensor("sp", [E_PAD, S_EL], bf16, kind="ExternalInput").ap()
    d_d = nc.dram_tensor("dp", [E_PAD, D_EL], bf16, kind="ExternalInput").ap()
    o_d = nc.dram_tensor("out", [B, E_PAD * N_GP, 3], bf16, kind="ExternalOutput").ap()

    s_v = s_d.rearrange("(n p c) f -> n p (c f)", p=P, c=C)
    d_v = d_d.rearrange("(n p c) f -> n p (c f)", p=P, c=C)
    # per chunk: [p, b, (c g t)] so src/dst iterate (p, b, run)
    o_v = o_d.rearrange("b (n p c g) t -> n p b (c g t)", p=P, c=C, g=N_GP)

    with tile.TileContext(nc) as tc:
        with (
            tc.tile_pool(name="io", bufs=IO_BUFS) as io_pool,
            tc.tile_pool(name="mid", bufs=MID_BUFS) as mid_pool,
        ):
            for i in range(N_CHUNKS):
                S = io_pool.tile([P, C * S_EL], bf16, tag="S")
                D = io_pool.tile([P, C * D_EL], bf16, tag="D")
                nc.sync.dma_start(out=S[:], in_=s_v[i])
                nc.sync.dma_start(out=D[:], in_=d_v[i])

                Sv = S[:].rearrange("p (c g t k) -> p c g t k", c=C, g=N_GP, t=3)
                Dv = D[:].rearrange("p (c b t k) -> p c b t k", c=C, b=B, t=3)

                # per-b intermediates: products for b are consumed by b's
                # tree immediately; per-b tags give natural cross-b pipelining
                Tbs, U1bs, U2bs = [], [], []
                for b in range(B):
                    Tb = mid_pool.tile([P, C * 216], bf16, tag=f"T{b}")
                    Tbs.append(Tb)
                    db = Dv[:, :, b]                       # [p, c, t, k]
                    dbg = db[:, :, None, :, :].to_broadcast([P, C, N_GP, 3, N_EN])
                    Tbv = Tb[:].rearrange("p (c g t k) -> p c g t k", c=C, g=N_GP, t=3)
                    nc.vector.tensor_tensor(
                        out=Tbv, in0=Sv, in1=dbg, op=mybir.AluOpType.mult
                    )
                for b in range(B):
                    U1b = mid_pool.tile([P, C * 108], bf16, tag=f"U1{b}")
                    U1bs.append(U1b)
                    Tbv = Tbs[b][:].rearrange("p (c g t k) -> p c g t k", c=C, g=N_GP, t=3)
                    U1bv = U1b[:].rearrange("p (c g t k) -> p c g t k", c=C, g=N_GP, t=3)
                    eng = nc.gpsimd if b < POOL_L1 else nc.vector
                    eng.tensor_tensor(
                        out=U1bv,
                        in0=Tbv[:, :, :, :, 0:4],
                        in1=Tbv[:, :, :, :, 4:8],
                        op=mybir.AluOpType.add,
                    )
                for b in range(B):
                    U2b = mid_pool.tile([P, C * 54], bf16, tag=f"U2{b}")
                    U2bs.append(U2b)
                    U1bv = U1bs[b][:].rearrange("p (c g t k) -> p c g t k", c=C, g=N_GP, t=3)
                    U2bv = U2b[:].rearrange("p (c g t k) -> p c g t k", c=C, g=N_GP, t=3)
                    eng2 = nc.vector if b < DVE_L2 else nc.gpsimd
                    eng2.tensor_tensor(
                        out=U2bv,
                        in0=U1bv[:, :, :, :, 0:2],
                        in1=U1bv[:, :, :, :, 2:4],
                        op=mybir.AluOpType.add,
                    )
                O = io_pool.tile([P, B * C * 27], bf16, tag="O")
                # per-partition layout [b][c][g][t], viewed with (c, b, g) order
                # to match the U2 slices' iteration order
                Ocb = O[:].rearrange("p (b c g t) -> p c b g t", b=B, c=C, g=N_GP)

                TMP = mid_pool.tile([P, C * B * N_GP], bf16, tag="TMP")
                TMPv = TMP[:].rearrange("p (c b g) -> p c b g", c=C, b=B)

                # L3 + strain combine, per-b so each b's tail overlaps the
                # next b's tree work:
                #   xx = U2[t0,0]+U2[t0,1] ; yy = U2[t1,...] ; p2 = U2[t2,...]
                #   xy = p2 - 0.5 xx - 0.5 yy
                xx_o = Ocb[:, :, :, :, 0]
                yy_o = Ocb[:, :, :, :, 1]
                xy_o = Ocb[:, :, :, :, 2]
                for b in range(B):
                    U2bv = U2bs[b][:].rearrange(
                        "p (c g t k) -> p c g t k", c=C, g=N_GP, t=3
                    )
                    eng3 = nc.vector
                    eng3.tensor_tensor(
                        out=xx_o[:, :, b], in0=U2bv[:, :, :, 0, 0],
                        in1=U2bv[:, :, :, 0, 1], op=mybir.AluOpType.add,
                    )
                    eng3.tensor_tensor(
                        out=yy_o[:, :, b], in0=U2bv[:, :, :, 1, 0],
                        in1=U2bv[:, :, :, 1, 1], op=mybir.AluOpType.add,
                    )
                    eng3.tensor_tensor(
                        out=TMPv[:, :, b], in0=U2bv[:, :, :, 2, 0],
                        in1=U2bv[:, :, :, 2, 1], op=mybir.AluOpType.add,
                    )
                    nc.vector.scalar_tensor_tensor(
                        out=TMPv[:, :, b], in0=xx_o[:, :, b], scalar=-0.5,
                        in1=TMPv[:, :, b],
                        op0=mybir.AluOpType.mult, op1=mybir.AluOpType.add,
                    )
                    nc.vector.scalar_tensor_tensor(
                        out=xy_o[:, :, b], in0=yy_o[:, :, b], scalar=-0.5,
                        in1=TMPv[:, :, b],
                        op0=mybir.AluOpType.mult, op1=mybir.AluOpType.add,
                    )

                nc.sync.dma_start(
                    out=o_v[i],
                    in_=O[:].rearrange("p (b f) -> p b f", b=B),
                )

    nc.compile()
    return nc


def _get_program():
    global _compiled
    if _compiled is None:
        _compiled = _build_program()
    return _compiled


def _marshal_core(inputs, shpdx, elem_nodes, c):
    """Build the per-core bf16 S'/D' blocks."""
    sl = slice(c * E_CORE, (c + 1) * E_CORE)
    en = elem_nodes[sl]                                   # [E, 8]
    disp = inputs[:, en]                                  # [B, E, 8, 2]
    u = disp[..., 0]                                      # [B, E, 8]
    v = disp[..., 1]
    w = u + v
    # D'[e, b, t, k] with t = (u, v, w)
    dstk = np.stack([u, v, w], axis=2)                    # [B, E, 3, 8]
    dstk = dstk.transpose(1, 0, 2, 3)                     # [E, B, 3, 8]
    dpad = np.zeros((E_PAD, D_EL), BF16)
    dpad[:E_CORE] = dstk.reshape(E_CORE, D_EL).astype(BF16)

    sx = shpdx[sl]                                        # [E, 9, 8, 2]
    s0 = sx[..., 0]
    s1 = sx[..., 1]
    a = 0.5 * (s0 + s1)
    sstk = np.stack([s0, s1, a], axis=2)                  # [E, 9, 3, 8]
    spad = np.zeros((E_PAD, S_EL), BF16)
    spad[:E_CORE] = sstk.reshape(E_CORE, S_EL).astype(BF16)
    return {"sp": spad, "dp": dpad}


def kernel(inputs, shpdx, elem_nodes, _want_trace=False):
    nc = _get_program()

    in_maps = [
        _marshal_core(inputs, shpdx, elem_nodes, c) for c in range(N_CORES)
    ]

    core_ids = list(range(N_CORES))
    res = run_bass_kernel_spmd(nc, in_maps, core_ids, trace=_want_trace)

    outs = []
    for c in range(N_CORES):
        o = res.results[c]["out"]                         # [B, E_PAD*9, 3] bf16
        outs.append(np.asarray(o[:, :E_CORE * N_GP, :], np.float32))
    full = np.concatenate(outs, axis=1)                   # [B, N_ELEM*9, 3]
    if _want_trace:
        return full, res
    return full
